# revision 30
# baseline (speedup 1.0000x reference)
"""EquiformerV2 EdgeDegreeEmbedding kernel for 8x TRN2 NeuronCores.

Strategy (target-sharded, no collectives):
  - Each core owns 2048 consecutive target nodes and processes every edge whose
    target lands in its range.  Host sorts/pads edges into 128 chunks per core;
    each chunk = 16 consecutive nodes with all their edges packed into 128
    "slots" (pad slots carry nid=-1 and an all-zero one-hot column).
  - Device per chunk: gaussian edge features + 2-layer MLP (PE matmuls in
    transposed activation layout), spherical harmonics on DVE, then the
    segment-sum is ONE set of 7 PE matmuls per chunk using a sh-weighted
    block-"one-hot" moving operand (Stat).  Output accumulates in PSUM as
    [c=128, (lm,n)=784], gets rms-normalised in place and written transposed;
    the host permutes the final layout (layout only, no arithmetic).
  - b3 is required to be all-zeros (spec fill guarantees this); b1/b2/gamma are
    fully supported.

e-order: slot s of chunk k sits at free-column k*128+lane.
"""

import math
import os
import sys

import numpy as np

sys.path.insert(0, "/opt/trn_rl_repo")

# ---------------------------------------------------------------- constants
LMAX = 6
NUM_COEF = 49
C = 128
NUM_GAUSS = 600
CUTOFF = 5.0
AVG_DEGREE = 23.395238876342773
N_ATOMS = 16384
N_EDGES = 98304
ZMAX = 90
D_EDGE = NUM_GAUSS + 2 * C

N_CORES = 8
NODES_PER_CORE = N_ATOMS // N_CORES          # 2048
NODES_PER_CHUNK = 16
CHUNKS = NODES_PER_CORE // NODES_PER_CHUNK   # 128
SLOTS = 128                                   # slots (edges) per chunk
S = CHUNKS * SLOTS                            # 16384 slots per core
NGC = 5                                       # gauss k-chunks (640 rows)
GPAD = NGC * 128                              # 640
GROUP = 512                                   # edges per MLP group
NGROUPS = S // GROUP                          # 32
ACC_W = NUM_COEF * NODES_PER_CHUNK            # 784
ACC_PAD = 1024                                # psum tile cols (2 banks)
# per-l column offsets inside the acc psum tile; l=0..4 in bank 1 (cols<512),
# l=5,6 in bank 2 -- a matmul output may not cross a 2KB psum bank boundary.
ACC_OFF = [0, 16, 64, 144, 256, 512, 688]     # region l: ACC_OFF[l] + (2l+1)*16
R1_W = 400                                    # bank-1 payload cols (lm 0..24)
R2_LO, R2_W = 512, 384                        # bank-2 payload cols (lm 25..48)
MS_OFF = 896                                  # ms scratch cols in acc tile
SCB_OFF = 912                                 # scaleB scratch cols in acc tile

GDELTA = CUTOFF / (NUM_GAUSS - 1)
GCOEFF = -0.5 / (2.0 * GDELTA) ** 2           # ~ -1794

L_OF = np.repeat(np.arange(LMAX + 1), 2 * np.arange(LMAX + 1) + 1)

USE_PART_BCAST = os.environ.get("K_PART_BCAST", "1") == "1"
SIM_COMPAT = os.environ.get("K_SIM_COMPAT", "0") == "1"


# ------------------------------------------------- spherical-harmonic folding
def sh_plan():
    """Plan the device op-chain for sh with all constants folded.

    Device computes Ptil[(l,m)] where P[(l,m)] = k[(l,m)] * Ptil[(l,m)].
    Final sh column (l,m) must equal  F * trig * P(l,m)  with
    F = Nlm * (sqrt2 if m>0) / AVG_DEGREE, so the per-column const is
    fcol = F * k[(l,m)].
    Returns (k, fcol) dicts plus the c2 consts for 3-op recurrences.
    """
    k = {(0, 0): 1.0}
    for m in range(1, LMAX + 1):
        k[(m, m)] = -(2 * m - 1) * k[(m - 1, m - 1)]
    for m in range(LMAX):
        k[(m + 1, m)] = (2 * m + 1) * k[(m, m)]
    c2 = {}
    for m in range(LMAX + 1):
        for l in range(m + 2, LMAX + 1):
            a = (2 * l - 1) / (l - m)
            b = -(l + m - 1) / (l - m)
            k[(l, m)] = a * k[(l - 1, m)]
            c2[(l, m)] = b * k[(l - 2, m)] / k[(l, m)]
    fcol = {}
    for l in range(LMAX + 1):
        for m in range(l + 1):
            Nlm = math.sqrt(
                (2 * l + 1) / (4.0 * math.pi)
                * math.factorial(l - m) / math.factorial(l + m)
            )
            f = Nlm / AVG_DEGREE
            if m > 0:
                f *= math.sqrt(2.0)
            fcol[(l, m)] = f * k[(l, m)]
    return k, c2, fcol


def col_of(l, m_signed):
    """flattened (l,m) column index; m_signed in [-l, l]."""
    return l * l + l + m_signed


# ---------------------------------------------------------------- host prep
def host_prep(inputs):
    pos = np.asarray(inputs["pos"], np.float32)
    Z = np.asarray(inputs["atomic_numbers"]).astype(np.int64)
    ei = np.asarray(inputs["edge_index"]).astype(np.int64)
    W1 = np.asarray(inputs["W1"], np.float32)
    b1 = np.asarray(inputs["b1"], np.float32)
    W2 = np.asarray(inputs["W2"], np.float32)
    b2 = np.asarray(inputs["b2"], np.float32)
    W3 = np.asarray(inputs["W3"], np.float32)
    b3 = np.asarray(inputs["b3"], np.float32)
    gamma = np.asarray(inputs["gamma"], np.float32)
    sph = np.asarray(inputs["sphere_emb"], np.float32)
    semb = np.asarray(inputs["src_emb"], np.float32)
    temb = np.asarray(inputs["tgt_emb"], np.float32)

    if np.any(b3 != 0.0):
        raise NotImplementedError("kernel assumes b3 == 0 (spec fill=zeros)")

    src, tgt = ei[0], ei[1]
    order = np.argsort(tgt, kind="stable")
    s_src, s_tgt = src[order], tgt[order]

    # weight folds (weight prep only)
    W1g = np.zeros((GPAD, C), np.float32)
    W1g[:NUM_GAUSS] = W1[:NUM_GAUSS]
    A_src = (semb @ W1[NUM_GAUSS:NUM_GAUSS + C]).astype(np.float16)
    A_tgt = (temb @ W1[NUM_GAUSS + C:]).astype(np.float16)
    A_src_p = np.zeros((128, C), np.float16)
    A_src_p[:ZMAX] = A_src
    A_tgt_p = np.zeros((128, C), np.float16)
    A_tgt_p[:ZMAX] = A_tgt
    sph_p = np.zeros((128, C), np.float16)
    sph_p[:ZMAX] = sph.astype(np.float16)

    offs = np.linspace(0.0, CUTOFF, NUM_GAUSS).astype(np.float32)
    ocol = np.full((128, NGC), 1.0e4, np.float32)
    for gc in range(NGC):
        n = min(128, NUM_GAUSS - gc * 128)
        if n > 0:
            ocol[:n, gc] = offs[gc * 128: gc * 128 + n]

    iota16 = np.broadcast_to(np.arange(16, dtype=np.float32)[None, :], (128, 16)).copy()

    in_maps = []
    for core in range(N_CORES):
        n0 = core * NODES_PER_CORE
        lo = np.searchsorted(s_tgt, n0)
        hi = np.searchsorted(s_tgt, n0 + NODES_PER_CORE)
        ct, cs = s_tgt[lo:hi], s_src[lo:hi]

        # slot arrays
        e_src = np.zeros(S, np.int64)
        e_tgt = np.full(S, n0, np.int64)
        nid = np.full((SLOTS, CHUNKS), -1.0, np.float32)
        valid = np.zeros(S, bool)
        # boundaries of each 16-node window inside this core's sorted edges
        w_start = np.searchsorted(ct, n0 + np.arange(0, NODES_PER_CORE + 1, NODES_PER_CHUNK))
        for kk in range(CHUNKS):
            a, b = w_start[kk], w_start[kk + 1]
            cnt = b - a
            if cnt > SLOTS:
                raise NotImplementedError(
                    f"chunk degree {cnt} > {SLOTS}; repack needed")
            sl = kk * SLOTS
            e_src[sl:sl + cnt] = cs[a:b]
            e_tgt[sl:sl + cnt] = ct[a:b]
            valid[sl:sl + cnt] = True
            nid[:cnt, kk] = (ct[a:b] - n0 - kk * NODES_PER_CHUNK).astype(np.float32)

        ps = pos[e_src]              # [S,3]
        pt = pos[e_tgt]
        ps[~valid] = 0.0
        pt[~valid] = 0.0
        zs = Z[e_src]
        zt = Z[e_tgt]

        # ptile [128, 6*CHUNKS]: col v*CHUNKS+chunk, partition=lane
        ptile = np.zeros((SLOTS, 6 * CHUNKS), np.float32)
        for v in range(3):
            ptile[:, v * CHUNKS:(v + 1) * CHUNKS] = ps[:, v].reshape(CHUNKS, SLOTS).T
            ptile[:, (3 + v) * CHUNKS:(4 + v) * CHUNKS] = pt[:, v].reshape(CHUNKS, SLOTS).T

        ohs = np.zeros((128, S), np.float16)
        oht = np.zeros((128, S), np.float16)
        sidx = np.arange(S)
        ohs[zs[sidx], sidx] = valid.astype(np.float16)
        oht[zt[sidx], sidx] = valid.astype(np.float16)

        ohn = np.zeros((128, NODES_PER_CORE), np.float16)
        nodes = np.arange(NODES_PER_CORE)
        ohn[Z[n0 + nodes], nodes] = 1.0

        # mask [128slot, CHUNKS*16]: col k*16+n == 1 iff slot belongs to node n
        maskb = (nid[:, :, None] ==
                 np.arange(16, dtype=np.float32)[None, None, :]).astype(np.float32)
        maskb = maskb.reshape(SLOTS, CHUNKS * 16)

        in_maps.append({
            "ptile": ptile,
            "ohs": ohs,
            "oht": oht,
            "ohn": ohn,
            "maskb": maskb,                    # [128, CHUNKS*16]
            "gammar": gamma.reshape(1, C).copy(),
            "onesc": np.ones((128, 1), np.float32),
            "w1g": W1g.reshape(NGC, 128, C).transpose(1, 0, 2).reshape(128, NGC * C).copy(),
            "asrc": A_src_p,
            "atgt": A_tgt_p,
            "spht": sph_p,
            "w2": W2.copy(),
            "w3m": W3.copy(),
            "b1c": b1.reshape(C, 1).copy(),
            "b2c": b2.reshape(C, 1).copy(),
            "ocol": ocol,
        })
    return in_maps


# w1g layout note: w1g[:, gc*128:(gc+1)*128] must be W1g[gc*128:(gc+1)*128, :]
# i.e. lhsT [K=g(128), M=c(128)] per gauss chunk.  The reshape above does that:
# W1g.reshape(NGC,128,C)[gc] == W1g[gc*128:(gc+1)*128].


# ---------------------------------------------------------------- device prog
def build_program(chunks=CHUNKS):
    import concourse.bass as bass
    import concourse.bacc as bacc
    import concourse.mybir as mybir
    from concourse.tile import TileContext

    fp32 = mybir.dt.float32
    fp16 = mybir.dt.float16
    bf16 = mybir.dt.bfloat16
    AF = mybir.ActivationFunctionType
    OP = mybir.AluOpType
    AX = mybir.AxisListType

    s = chunks * SLOTS
    group = min(GROUP, s)
    ngroups = s // group

    _, c2const, fcol = sh_plan()

    nc = bacc.Bacc()
    inp = {}
    for name, shape, dt in [
        ("ptile", [128, 6 * CHUNKS], fp32),
        ("ohs", [128, S], fp16),
        ("oht", [128, S], fp16),
        ("ohn", [128, NODES_PER_CORE], fp16),
        ("maskb", [128, CHUNKS * 16], fp32),
        ("gammar", [1, C], fp32),
        ("onesc", [128, 1], fp32),
        ("w1g", [128, NGC * C], fp32),
        ("asrc", [128, C], fp16),
        ("atgt", [128, C], fp16),
        ("spht", [128, C], fp16),
        ("w2", [C, C], fp32),
        ("w3m", [C, 7 * C], fp32),
        ("b1c", [C, 1], fp32),
        ("b2c", [C, 1], fp32),
        ("ocol", [128, NGC], fp32),
    ]:
        inp[name] = nc.dram_tensor(name, shape, dt, kind="ExternalInput")
    out_d = nc.dram_tensor("out", [128, chunks * ACC_W], fp32, kind="ExternalOutput")
    drow_d = nc.dram_tensor("drow_scratch", [1, S], fp32)

    with TileContext(nc) as tc:
        ctx_pools = []

        const = tc.alloc_tile_pool(name="const", bufs=1)
        sh_pool = tc.alloc_tile_pool(name="sh", bufs=1)
        tmp_pool = tc.alloc_tile_pool(name="shtmp", bufs=1)
        ctx_pools += [const, sh_pool, tmp_pool]

        def load(name, shape=None, dt=fp32):
            t = inp[name]
            tile = const.tile(list(t.shape), t.dtype, name=f"L_{name}")
            nc.sync.dma_start(out=tile[:], in_=t[:])
            return tile

        ptile = load("ptile")
        ohn = load("ohn")
        maskb = load("maskb")
        gammar = load("gammar")
        onesc = load("onesc")
        w1g = load("w1g")
        asrc = load("asrc")
        atgt = load("atgt")
        spht = load("spht")
        w2 = load("w2")
        w3m = load("w3m")
        b1c = load("b1c")
        b2c = load("b2c")
        ocol = load("ocol")

        CH = chunks
        V = nc.vector
        A = nc.scalar
        G = nc.gpsimd
        T = nc.tensor

        def tt(name, in0, in1, op):
            o = tmp_pool.tile([128, CH], fp32, name=name, tag=name)
            V.tensor_tensor(o[:], in0, in1, op)
            return o

        def ts(name, in0, s1, op1, s2=None, op2=None):
            o = tmp_pool.tile([128, CH], fp32, name=name, tag=name)
            if s2 is None:
                V.tensor_scalar(o[:], in0, s1, None, op1)
            else:
                V.tensor_scalar(o[:], in0, s1, s2, op1, op2)
            return o

        def pcol(v, ch=None):
            ap = ptile[:, v * CHUNKS:v * CHUNKS + CH]
            return ap

        eps12 = const.tile([128, 1], fp32, name="eps12")
        V.memset(eps12[:], 1.0e-12)

        # ---- dirs / dist ------------------------------------------------
        dx = tt("dx", pcol(0), pcol(3), OP.subtract)
        dy = tt("dy", pcol(1), pcol(4), OP.subtract)
        dz = tt("dz", pcol(2), pcol(5), OP.subtract)
        xx = tt("xx", dx[:], dx[:], OP.mult)
        yy = tt("yy", dy[:], dy[:], OP.mult)
        zz = tt("zz", dz[:], dz[:], OP.mult)
        d2a = tt("d2a", xx[:], yy[:], OP.add)
        d2 = tt("d2", d2a[:], zz[:], OP.add)
        dcol = tmp_pool.tile([128, CH], fp32, name="dcol")
        A.activation(dcol[:], d2[:], AF.Sqrt, bias=eps12[:, 0:1])
        rd = tmp_pool.tile([128, CH], fp32, name="rd")
        V.reciprocal(rd[:], dcol[:])
        ux = tt("ux", dx[:], rd[:], OP.mult)
        uy = tt("uy", dy[:], rd[:], OP.mult)
        uz = tt("uz", dz[:], rd[:], OP.mult)

        # d -> DRAM scratch row in e-order (chunk*128+lane); issued from the
        # ACT engine so the dcol dependency is same-engine (no sem wait --
        # DMA instructions may carry at most one sync wait).
        A.dma_start(
            out=drow_d[0:1, 0:s].rearrange("p (c l) -> p l c", c=CH),
            in_=dcol[:],
        )

        ct = ts("ct", uz[:], -1.0, OP.max, 1.0, OP.min)
        ct2 = tt("ct2", ct[:], ct[:], OP.mult)
        st2a = ts("st2a", ct2[:], -1.0, OP.mult, 1.0, OP.add)
        st2 = ts("st2", st2a[:], 1.0e-12, OP.max)
        st = tmp_pool.tile([128, CH], fp32, name="st")
        A.activation(st[:], st2[:], AF.Sqrt)

        r2a = tt("r2a", ux[:], ux[:], OP.mult)
        r2b = tt("r2b", uy[:], uy[:], OP.mult)
        rho2 = tt("rho2", r2a[:], r2b[:], OP.add)
        isz = ts("isz", rho2[:], 0.0, OP.is_equal)
        rho2g = tt("rho2g", rho2[:], isz[:], OP.add)
        rho = tmp_pool.tile([128, CH], fp32, name="rho")
        A.activation(rho[:], rho2g[:], AF.Sqrt)
        rrho = tmp_pool.tile([128, CH], fp32, name="rrho")
        V.reciprocal(rrho[:], rho[:])
        c1a = tt("c1a", ux[:], rrho[:], OP.mult)
        c1 = tt("c1", c1a[:], isz[:], OP.add)      # cos(phi), =1 at rho=0
        s1 = tt("s1", uy[:], rrho[:], OP.mult)     # sin(phi), =0 at rho=0

        # trig recurrences
        cm = {1: c1}
        sm = {1: s1}
        for m in range(2, LMAX + 1):
            a1 = tt(f"ca{m}", c1[:], cm[m - 1][:], OP.mult)
            a2 = tt(f"cb{m}", s1[:], sm[m - 1][:], OP.mult)
            cm[m] = tt(f"cm{m}", a1[:], a2[:], OP.subtract)
            a3 = tt(f"sa{m}", s1[:], cm[m - 1][:], OP.mult)
            a4 = tt(f"sb{m}", c1[:], sm[m - 1][:], OP.mult)
            sm[m] = tt(f"sm{m}", a3[:], a4[:], OP.add)

        # P-tilde chains
        P = {}
        P[(1, 1)] = st
        for m in range(2, LMAX + 1):
            P[(m, m)] = tt(f"p{m}{m}", st[:], P[(m - 1, m - 1)][:], OP.mult)
        P[(1, 0)] = ct
        for m in range(1, LMAX):
            P[(m + 1, m)] = tt(f"p{m+1}{m}", ct[:], P[(m, m)][:], OP.mult)
        # l = m+2 .. 6
        for m in range(0, LMAX - 1):
            for l in range(m + 2, LMAX + 1):
                if m == 0 and l == 2:
                    t1 = tt("p20a", ct[:], ct[:], OP.mult)
                    P[(2, 0)] = ts("p20", t1[:], c2const[(2, 0)], OP.add)
                else:
                    t1 = tt(f"pa{l}{m}", ct[:], P[(l - 1, m)][:], OP.mult)
                    t2 = ts(f"pb{l}{m}", P[(l - 2, m)][:], c2const[(l, m)], OP.mult)
                    P[(l, m)] = tt(f"p{l}{m}", t1[:], t2[:], OP.add)

        # assemble sh_all [128, 49*CH], col = lm*CH + chunk
        sh_all = sh_pool.tile([128, NUM_COEF * CH], fp32, name="sh_all")

        def shcol(lm):
            return sh_all[:, lm * CH:(lm + 1) * CH]

        # m = 0 columns
        one_t = None
        for l in range(LMAX + 1):
            if l == 0:
                # col = f * Ptil(0,0) = f * 1
                V.memset(shcol(col_of(0, 0)), float(fcol[(0, 0)]))
            else:
                V.tensor_scalar(shcol(col_of(l, 0)), P[(l, 0)][:],
                                float(fcol[(l, 0)]), None, OP.mult)
        # m > 0 columns
        for l in range(1, LMAX + 1):
            for m in range(1, l + 1):
                pscaled = ts(f"pf{l}{m}", P[(l, m)][:], float(fcol[(l, m)]), OP.mult)
                V.tensor_tensor(shcol(col_of(l, m)), cm[m][:], pscaled[:], OP.mult)
                V.tensor_tensor(shcol(col_of(l, -m)), sm[m][:], pscaled[:], OP.mult)

        # ---- streaming pools -------------------------------------------
        epool = tc.alloc_tile_pool(name="E", bufs=2)
        ohpool = tc.alloc_tile_pool(name="oh", bufs=2)
        hpool = tc.alloc_tile_pool(name="hT", bufs=2)
        radpool = tc.alloc_tile_pool(name="radS", bufs=2)
        statpool = tc.alloc_tile_pool(name="stat", bufs=2)
        sqpool = tc.alloc_tile_pool(name="sq", bufs=2)
        outpool = tc.alloc_tile_pool(name="outS", bufs=2)
        smallpool = tc.alloc_tile_pool(name="small", bufs=3)
        dbpool = tc.alloc_tile_pool(name="dbc", bufs=2)
        ctx_pools += [epool, ohpool, hpool, radpool, statpool, sqpool, outpool,
                      smallpool, dbpool]

        acc_pool = tc.alloc_tile_pool(name="accp", bufs=2, space="PSUM")
        mlp_pool = tc.alloc_tile_pool(name="mlpp", bufs=2, space="PSUM")
        rad_pool = tc.alloc_tile_pool(name="radp", bufs=1, space="PSUM")
        ctx_pools += [acc_pool, mlp_pool, rad_pool]

        for g in range(ngroups):
            off = g * group
            ohs_t = ohpool.tile([128, group], fp16, name="ohs_t", tag="ohs")
            oht_t = ohpool.tile([128, group], fp16, name="oht_t", tag="oht")
            nc.sync.dma_start(out=ohs_t[:], in_=inp["ohs"][:, off:off + group])
            nc.sync.dma_start(out=oht_t[:], in_=inp["oht"][:, off:off + group])

            db = dbpool.tile([128, group], fp32, name="db", tag="db")
            nc.sync.dma_start(
                out=db[:],
                in_=drow_d[0:1, off:off + group].broadcast_to([128, group]),
            )
            din = db[:]

            h1p = mlp_pool.tile([128, group], fp32, name="h1p", tag="mlp")
            Es = []
            for gc in range(NGC):
                E = epool.tile([128, group], fp32, name=f"E{gc}", tag=f"E{gc}")
                V.tensor_scalar(E[:], din, ocol[:, gc:gc + 1], None, OP.subtract)
                G.tensor_tensor(E[:], E[:], E[:], OP.mult)
                A.activation(E[:], E[:], AF.Exp, scale=float(GCOEFF))
                Es.append(E)
            for gc in range(NGC):
                T.matmul(h1p[:], lhsT=w1g[:, gc * C:(gc + 1) * C], rhs=Es[gc][:],
                         start=(gc == 0), stop=False)
            T.matmul(h1p[:], lhsT=asrc[:], rhs=ohs_t[:], start=False, stop=False)
            T.matmul(h1p[:], lhsT=atgt[:], rhs=oht_t[:], start=False, stop=True)

            def silu_evict(dst, src_psum, bias_col, nm):
                if not SIM_COMPAT:
                    A.activation(dst, src_psum, AF.Silu, bias=bias_col)
                else:
                    sg = hpool.tile([128, group], fp32, name=f"sg{nm}", tag=f"sg{nm}")
                    A.activation(sg[:], src_psum, AF.Sigmoid, bias=bias_col)
                    xb = hpool.tile([128, group], fp32, name=f"xb{nm}", tag=f"xb{nm}")
                    V.tensor_scalar(xb[:], src_psum, bias_col, None, OP.add)
                    V.tensor_tensor(dst, xb[:], sg[:], OP.mult)

            h1T = hpool.tile([128, group], fp32, name="h1T", tag="h1T")
            silu_evict(h1T[:], h1p[:], b1c[:, 0:1], "1")

            h2p = mlp_pool.tile([128, group], fp32, name="h2p", tag="mlp")
            T.matmul(h2p[:], lhsT=w2[:], rhs=h1T[:], start=True, stop=True)
            h2T = hpool.tile([128, group], fp32, name="h2T", tag="h2T")
            silu_evict(h2T[:], h2p[:], b2c[:, 0:1], "2")

            for ci in range(group // SLOTS):
                k = g * (group // SLOTS) + ci
                lhs_h2 = h2T[:, ci * SLOTS:(ci + 1) * SLOTS]
                rada = rad_pool.tile([128, 512], fp32, name="rada", tag="rada")
                radb = rad_pool.tile([128, 384], fp32, name="radb", tag="radb")
                T.matmul(rada[:], lhsT=lhs_h2, rhs=w3m[:, 0:512], start=True, stop=True)
                T.matmul(radb[:], lhsT=lhs_h2, rhs=w3m[:, 512:896], start=True, stop=True)
                radS = radpool.tile([128, 7 * C], fp32, name="radS", tag="radS")
                V.tensor_copy(radS[:, 0:512], rada[:])
                A.activation(radS[:, 512:896], radb[:], AF.Copy)

                stat = statpool.tile([128, ACC_W], fp32, name="stat", tag="stat")
                G.tensor_tensor(
                    stat[:].rearrange("p (l n) -> p l n", n=16),
                    maskb[:, k * 16:(k + 1) * 16].unsqueeze(1).broadcast_to(
                        [128, NUM_COEF, 16]),
                    sh_all[:, k:NUM_COEF * CH:CH].unsqueeze(2).broadcast_to(
                        [128, NUM_COEF, 16]),
                    OP.mult,
                )

                acc = acc_pool.tile([128, ACC_PAD], fp32, name="acc", tag="acc")
                for l in range(LMAX + 1):
                    s0 = l * l * 16
                    s1 = (l * l + 2 * l + 1) * 16
                    a0 = ACC_OFF[l]
                    T.matmul(acc[:, a0:a0 + (s1 - s0)],
                             lhsT=radS[:, l * C:(l + 1) * C],
                             rhs=stat[:, s0:s1],
                             start=True, stop=(l != 0))
                    if l == 0:
                        T.matmul(acc[:, 0:16], lhsT=spht[:],
                                 rhs=ohn[:, k * 16:(k + 1) * 16],
                                 start=False, stop=True)

                sq = sqpool.tile([128, ACC_W], bf16, name="sq", tag="sq")
                A.activation(sq[:, 0:R1_W], acc[:, 0:R1_W], AF.Square)
                A.activation(sq[:, R1_W:ACC_W], acc[:, R2_LO:R2_LO + R2_W],
                             AF.Square)
                msqa = smallpool.tile([128, 16], fp32, name="msqa", tag="msqa")
                msqb = smallpool.tile([128, 16], fp32, name="msqb", tag="msqb")
                V.tensor_reduce(
                    msqa[:].unsqueeze(2),
                    sq[:, 0:R1_W].rearrange("p (l n) -> p n l", n=16),
                    AX.X, OP.add,
                )
                V.tensor_reduce(
                    msqb[:].unsqueeze(2),
                    sq[:, R1_W:ACC_W].rearrange("p (l n) -> p n l", n=16),
                    AX.X, OP.add,
                )
                msq = smallpool.tile([128, 16], fp32, name="msq", tag="msq")
                V.tensor_tensor(msq[:], msqa[:], msqb[:], OP.add)
                T.matmul(acc[0:1, MS_OFF:MS_OFF + 16], lhsT=onesc[:], rhs=msq[:],
                         start=True, stop=True)
                msr = smallpool.tile([1, 16], fp32, name="msr", tag="msr")
                V.tensor_scalar(msr[:], acc[0:1, MS_OFF:MS_OFF + 16],
                                1.0 / 6272.0, 1.0e-6, OP.mult, OP.add)
                sqr = smallpool.tile([1, 16], fp32, name="sqr", tag="sqr")
                A.activation(sqr[:], msr[:], AF.Sqrt)
                scr = smallpool.tile([1, 16], fp32, name="scr", tag="scr")
                V.reciprocal(scr[:], sqr[:])
                T.matmul(acc[:, SCB_OFF:SCB_OFF + 16], lhsT=gammar[:], rhs=scr[:],
                         start=True, stop=True)
                scB = smallpool.tile([128, 16], fp32, name="scB", tag="scB")
                V.tensor_copy(scB[:], acc[:, SCB_OFF:SCB_OFF + 16])

                outS = outpool.tile([128, ACC_W], fp32, name="outS", tag="outS")
                V.tensor_tensor(
                    outS[:, 0:R1_W].rearrange("p (l n) -> p l n", n=16),
                    acc[:, 0:R1_W].rearrange("p (l n) -> p l n", n=16),
                    scB[:].unsqueeze(1).broadcast_to([128, 25, 16]),
                    OP.mult,
                )
                V.tensor_tensor(
                    outS[:, R1_W:ACC_W].rearrange("p (l n) -> p l n", n=16),
                    acc[:, R2_LO:R2_LO + R2_W].rearrange("p (l n) -> p l n", n=16),
                    scB[:].unsqueeze(1).broadcast_to([128, 24, 16]),
                    OP.mult,
                )
                nc.sync.dma_start(out=out_d[:, k * ACC_W:(k + 1) * ACC_W],
                                  in_=outS[:])

        for p in reversed(ctx_pools):
            p.release()
    nc.compile()
    return nc


LAST_RESULTS = None


def kernel(**inputs):
    global LAST_RESULTS
    from concourse.bass_utils import run_bass_kernel_spmd

    in_maps = host_prep(inputs)
    nc = build_program()
    kw = {}
    if os.environ.get("K_TRACE", "0") == "1":
        kw = dict(trace=True, trace_cores=[0])
    res = run_bass_kernel_spmd(nc, in_maps, core_ids=list(range(N_CORES)), **kw)
    LAST_RESULTS = res
    out = np.empty((N_ATOMS, NUM_COEF, C), np.float32)
    for core in range(N_CORES):
        o = res.results[core]["out"]                    # [128, CHUNKS*784]
        o = o.reshape(C, CHUNKS, NUM_COEF, NODES_PER_CHUNK)
        o = o.transpose(1, 3, 2, 0).reshape(NODES_PER_CORE, NUM_COEF, C)
        out[core * NODES_PER_CORE:(core + 1) * NODES_PER_CORE] = o
    return out
